# revision 28
# baseline (speedup 1.0000x reference)
import math

import numpy as np

B, H_FULL, S, D = 2, 16, 2048, 64
P = 128
NT = S // P
HPC = 4
NCORES = 8
HALF = 1024
SCALE = 1.0 / math.sqrt(D)
M1 = D + 1

TUNE = {
    "ptp_bufs": 22,
    "qk_bufs": 2,
    "stage_bufs": 1,
    "psS_bufs": 2,
    "psT_bufs": 2,
    "pn_bufs": 4,
    "act_drain_every": 5,
    "b_pump": 4,
    "ablate": "",
}

_BUILD_CACHE: dict = {}


def _classify_blocks(mask_u8: np.ndarray):
    nb = S // P
    u = mask_u8.any(axis=0)
    blk_any = u.reshape(nb, P, nb, P).any(axis=(1, 3))

    tril = np.tril(np.ones((S, S), np.uint8))
    if (mask_u8 == tril[None]).all():
        mode = "causal"
    elif mask_u8.all():
        mode = "none"
    else:
        mode = "general"

    p2_sched = []
    for half in range(2):
        q0, q1 = half * HALF, (half + 1) * HALF
        jlo, jhi = q0 // P, q1 // P
        kis = []
        for ki in range(nb):
            if not blk_any[jlo:jhi, ki].any():
                continue
            qs = max(q0, ki * P) if mode == "causal" else q0
            kis.append((ki, qs))
        if kis:
            kis[0] = (kis[0][0], q0)
        p2_sched.append(kis)
    return mode, blk_any, p2_sched


def _build(mode, blk_any, p2_sched):
    from concourse import bacc
    import concourse.mybir as mybir
    import concourse.tile as tile

    f32 = mybir.dt.float32
    f32r = mybir.dt.float32r
    u8 = mybir.dt.uint8
    Exp = mybir.ActivationFunctionType.Exp
    Copy = mybir.ActivationFunctionType.Copy

    nc = bacc.Bacc()

    qT_d = nc.dram_tensor("qT", [HPC, D, S], f32, kind="ExternalInput")
    kT_d = nc.dram_tensor("kT", [HPC, D, S], f32, kind="ExternalInput")
    vo_d = nc.dram_tensor("vo", [HPC // 2, S, 2, M1], f32, kind="ExternalInput")
    ident_d = nc.dram_tensor("ident", [P, P], f32, kind="ExternalInput")
    validkq_d = nc.dram_tensor("validkq", [P, P], u8, kind="ExternalInput")
    if mode == "general":
        m8T_d = nc.dram_tensor("m8T", [S, S], u8, kind="ExternalInput")
    w_out = nc.dram_tensor("w_out", [HPC, S, S], f32, kind="ExternalOutput")
    o_out = nc.dram_tensor("o_out", [HPC, S, D], f32, kind="ExternalOutput")

    half_jsets = [set(ki for ki, _ in p2_sched[h]) for h in range(2)]
    ablate = TUNE["ablate"]

    with tile.TileContext(nc) as tc:
        with (
            tc.tile_pool(name="consts", bufs=1) as consts,
            tc.tile_pool(name="stage", bufs=TUNE["stage_bufs"]) as stage,
            tc.tile_pool(name="qk", bufs=TUNE["qk_bufs"]) as qk,
            tc.tile_pool(name="ptp", bufs=TUNE["ptp_bufs"]) as ptp,
            tc.tile_pool(name="work", bufs=4) as work,
            tc.tile_pool(name="pnp", bufs=TUNE["pn_bufs"]) as pnp,
            tc.tile_pool(name="small", bufs=4) as small,
            tc.tile_pool(name="outp", bufs=2) as outp,
            tc.tile_pool(name="psS", bufs=TUNE["psS_bufs"], space="PSUM") as psS,
            tc.tile_pool(name="psO", bufs=1, space="PSUM") as psO,
            tc.tile_pool(name="psT", bufs=TUNE["psT_bufs"], space="PSUM") as psT,
        ):
            identf = consts.tile([P, P], f32)
            nc.sync.dma_start(identf[:], ident_d[:])
            identr = consts.tile([P, P], f32r)
            nc.vector.tensor_copy(identr[:], identf[:])
            validkq = consts.tile([P, P], u8)
            nc.sync.dma_start(validkq[:], validkq_d[:])

            drain_ctr = [0]
            bq: list = []

            def _plan_phase_b(i, half, pt_map, h, recips):
                if "B" in ablate:
                    return
                js = [
                    j
                    for j in range(NT)
                    if j in half_jsets[half] and blk_any[i][j]
                ]
                if not js:
                    return
                runs = []
                run = [js[0]]
                for j in js[1:]:
                    if j == run[-1] + 1:
                        run.append(j)
                    else:
                        runs.append(run)
                        run = [j]
                runs.append(run)
                for run in runs:
                    groups = [run[g0 : g0 + 4] for g0 in range(0, len(run), 4)]
                    bq.append([i, run, groups, dict(pt_map), h, None, recips])

            def _pump_b(budget):
                while budget > 0 and bq:
                    unit = bq[0]
                    i, run, groups, pmap, uh, pn, recips = unit
                    if pn is None:
                        pn = pnp.tile([P, NT, P], f32, tag="pn")
                        unit[5] = pn
                    grp = groups.pop(0)
                    ng = len(grp)
                    tgb = psT.tile([P, 4, P], f32r, tag="tp")
                    for gj, j in enumerate(grp):
                        ptj, qsj = pmap[j]
                        nc.tensor.transpose(
                            tgb[:, gj, :],
                            ptj[:, i * P - qsj : i * P - qsj + P],
                            identr[:],
                        )
                    dst = pn[:, grp[0] : grp[0] + ng, :]
                    src = tgb[:, 0:ng, :].bitcast(f32)
                    drain_ctr[0] += 1
                    rscale = 1.0 if "R" in ablate else recips[:, i : i + 1]
                    if drain_ctr[0] % TUNE["act_drain_every"] == 0:
                        nc.scalar.activation(dst, src, Copy, scale=rscale)
                    else:
                        nc.vector.tensor_scalar_mul(dst, src, rscale)
                    budget -= 1
                    if not groups:
                        bq.pop(0)
                        j0, j1 = run[0], run[-1]
                        if "W" not in ablate:
                            nc.sync.dma_start(
                                w_out[
                                    uh,
                                    i * P : (i + 1) * P,
                                    j0 * P : (j1 + 1) * P,
                                ].rearrange("p (j c) -> p j c", j=j1 - j0 + 1),
                                pn[:, j0 : j1 + 1, :],
                            )

            for h in range(HPC):
                qraw = stage.tile([D, S], f32, tag="qraw")
                kraw = stage.tile([D, S], f32, tag="kraw")
                for c0 in range(0, S, HALF):
                    c1 = c0 + HALF
                    nc.sync.dma_start(kraw[:, c0:c1], kT_d[h, :, c0:c1])
                    nc.sync.dma_start(qraw[:, c0:c1], qT_d[h, :, c0:c1])
                if h % 2 == 0:
                    voraw2 = stage.tile([P, NT, 2, M1], f32, tag="voraw")
                    nc.sync.dma_start(
                        voraw2[:],
                        vo_d[h // 2].rearrange("(c p) j d -> p c j d", p=P),
                    )
                    vor2 = qk.tile([P, NT, 2, M1], f32r, tag="vor")
                    nc.gpsimd.tensor_copy(vor2[:], voraw2[:])
                qtr = qk.tile([D, S], f32r, tag="qtr")
                ktr = qk.tile([D, S], f32r, tag="ktr")
                for c0 in range(0, S, HALF):
                    c1 = c0 + HALF
                    nc.gpsimd.tensor_copy(ktr[:, c0:c1], kraw[:, c0:c1])
                    nc.gpsimd.tensor_copy(qtr[:, c0:c1], qraw[:, c0:c1])

                recips = small.tile([P, NT], f32, tag="recips")

                for half in range(2):
                    q0 = half * HALF
                    q1 = q0 + HALF
                    kis = p2_sched[half]
                    osb = outp.tile([P, 8, D], f32, tag="osb")
                    if not kis:
                        nc.vector.memset(osb[:], 0.0)
                        nc.sync.dma_start(
                            o_out[h, q0:q1, :].rearrange(
                                "(j p) d -> p j d", p=P
                            ),
                            osb[:],
                        )
                        continue

                    pt_map = {}
                    pso = psO.tile([M1, HALF], f32, tag="pso")
                    osb_t = outp.tile([M1, HALF], f32, tag="osb_t")
                    nki = len(kis)
                    final_after = [[] for _ in range(nki)]
                    for jj in range(8):
                        qb = q0 + jj * P
                        last = max(
                            idx for idx, (ki, qs) in enumerate(kis) if qs <= qb
                        )
                        final_after[last].append(jj)

                    for idx, (ki, qs) in enumerate(kis):
                        w = q1 - qs
                        pt = ptp.tile([P, HALF], f32r, tag="pt")
                        for n0 in range(0, w, 1024):
                            n1 = min(w, n0 + 1024)
                            cw = n1 - n0
                            ps = psS.tile([P, 1024], f32, tag="ps")
                            for m0 in range(0, cw, 512):
                                m1 = min(cw, m0 + 512)
                                nc.tensor.matmul(
                                    ps[:, m0:m1],
                                    ktr[:, ki * P : (ki + 1) * P],
                                    qtr[:, qs + n0 + m0 : qs + n0 + m1],
                                    start=True,
                                    stop=True,
                                )
                            nc.scalar.activation(
                                pt[:, n0:n1], ps[:, 0:cw], Exp, scale=SCALE
                            )
                        if mode == "causal" and qs == ki * P:
                            nc.gpsimd.tensor_mul(
                                pt[:, 0:P], pt[:, 0:P], validkq[:]
                            )
                        elif mode == "general":
                            mrowt = work.tile([P, HALF], u8, tag="mrowt")
                            nc.sync.dma_start(
                                mrowt[:, 0:w],
                                m8T_d[ki * P : (ki + 1) * P, qs:q1],
                            )
                            nc.gpsimd.tensor_mul(
                                pt[:, 0:w], pt[:, 0:w], mrowt[:, 0:w]
                            )
                        pieces = [
                            (n0, min(w, n0 + 512))
                            for n0 in range(0, w, 512)
                        ]
                        for n0, n1 in pieces:
                            nc.tensor.matmul(
                                pso[:, (qs - q0) + n0 : (qs - q0) + n1],
                                vor2[:, ki, h % 2, :],
                                pt[:, n0:n1],
                                start=(idx == 0),
                                stop=(idx == nki - 1),
                                skip_group_check=True,
                            )
                        pt_map[ki] = (pt, qs)

                        if "O" not in ablate:
                            for jj in final_after[idx]:
                                c0 = jj * P
                                nc.vector.tensor_copy(
                                    osb_t[:, c0 : c0 + P],
                                    pso[:, c0 : c0 + P],
                                )
                                tg = psT.tile([P, 4, P], f32, tag="tp")
                                nc.tensor.transpose(
                                    tg[:, 0, 0:M1],
                                    osb_t[:, c0 : c0 + P],
                                    identf[0:M1, 0:M1],
                                )
                                qi = half * 8 + jj
                                nc.vector.reciprocal(
                                    recips[:, qi : qi + 1],
                                    tg[:, 0, D : D + 1],
                                )
                                nc.vector.tensor_scalar_mul(
                                    osb[:, jj, :],
                                    tg[:, 0, 0:D],
                                    recips[:, qi : qi + 1],
                                )
                        for jj in final_after[idx]:
                            _plan_phase_b(
                                half * 8 + jj, half, pt_map, h, recips
                            )
                        _pump_b(TUNE["b_pump"])

                    if "O" in ablate:
                        nc.vector.memset(
                            recips[:, half * 8 : half * 8 + 8], 1.0
                        )
                    else:
                        nc.sync.dma_start(
                            o_out[h, q0:q1, :].rearrange(
                                "(j p) d -> p j d", p=P
                            ),
                            osb[:],
                        )
                _pump_b(10**9)

    nc.compile()
    return nc


def _get_nc(mask_u8: np.ndarray):
    mode, blk_any, p2_sched = _classify_blocks(mask_u8)
    key = (
        mode,
        blk_any.tobytes(),
        tuple(tuple(x) for x in p2_sched),
        tuple(sorted((k, v) for k, v in TUNE.items())),
    )
    if key not in _BUILD_CACHE:
        _BUILD_CACHE[key] = _build(mode, blk_any, p2_sched)
    return mode, _BUILD_CACHE[key]


def kernel(queries, keys, values, mask, _trace=False):
    from concourse.bass_utils import run_bass_kernel_spmd

    queries = np.asarray(queries, dtype=np.float32)
    keys = np.asarray(keys, dtype=np.float32)
    values = np.asarray(values, dtype=np.float32)
    mask_u8 = np.ascontiguousarray(np.asarray(mask)).astype(np.uint8)

    mode, nc = _get_nc(mask_u8)

    qT = np.ascontiguousarray(queries.transpose(0, 1, 3, 2))
    kT = np.ascontiguousarray(keys.transpose(0, 1, 3, 2))
    vo = np.concatenate(
        [values, np.ones(values.shape[:-1] + (1,), np.float32)], axis=-1
    )
    vo = np.ascontiguousarray(
        vo.reshape(B, H_FULL // 2, 2, S, M1).transpose(0, 1, 3, 2, 4)
    )

    ident = np.eye(P, dtype=np.float32)
    tril128 = np.tril(np.ones((P, P), np.uint8))
    validkq = np.ascontiguousarray(tril128.T)

    in_maps = []
    for c in range(NCORES):
        b = c // (NCORES // B)
        h0 = (c % (NCORES // B)) * HPC
        m = {
            "qT": np.ascontiguousarray(qT[b, h0 : h0 + HPC]),
            "kT": np.ascontiguousarray(kT[b, h0 : h0 + HPC]),
            "vo": np.ascontiguousarray(vo[b, h0 // 2 : h0 // 2 + HPC // 2]),
            "ident": ident,
            "validkq": validkq,
        }
        if mode == "general":
            m["m8T"] = np.ascontiguousarray(mask_u8[b].transpose(1, 0))
        in_maps.append(m)

    res = run_bass_kernel_spmd(
        nc, in_maps, core_ids=list(range(NCORES)), trace=_trace
    )
    if _trace:
        global LAST_RESULTS
        LAST_RESULTS = res

    weights = np.empty((B, H_FULL, S, S), np.float32)
    output = np.empty((B, H_FULL, S, D), np.float32)
    for c in range(NCORES):
        b = c // (NCORES // B)
        h0 = (c % (NCORES // B)) * HPC
        weights[b, h0 : h0 + HPC] = res.results[c]["w_out"]
        output[b, h0 : h0 + HPC] = res.results[c]["o_out"]
    return (output, weights)
